# revision 22
# baseline (speedup 1.0000x reference)
"""Trainium2 Bass kernel for nn_ATTENTION_5549097746558.

Two-block transformer with time-relative attention
(aw = QK' + Q.aK + Q.timeK[tm]; out = A@(V+aV) + sum_t S[t]*timeV[t]).

Strategy: pure data parallelism over batch (B=16 over 8 cores, 2 each).
The time-embedding terms are computed entirely on-chip in the t-domain:
  * G[l,m] = QT[l, tm[l,m]]   (per-row gather) is realised as
    compress-scatter -> diff -> cumsum -> unsort-scatter, all built on
    GPSIMD local_scatter (per-partition permutation scatter) with
    host-precomputed index tensors, plus DVE prefix scans.
  * S[l,t] = sum_m A[l,m][tm[l,m]=t]  (per-row histogram) is realised
    as sort-scatter -> cumsum -> boundary-scatter -> running-max, and
    the final contraction sum_t S[t] tV[t] is rewritten by summation
    by parts as sum_t R[t] (tV[t]-tV[t+1]) so S is never materialised.
All index tensors are pure functions of the integer inputs and are
prepared on the host; every FLOP on real data happens on the device.
"""
import sys

sys.path.insert(0, "/opt/trn_rl_repo")

import numpy as np

import bass_rust
import concourse.bacc as bacc
import concourse.mybir as mybir
from concourse import library_config
from concourse.bass_utils import run_bass_kernel_spmd
from concourse.tile import TileContext
from concourse.vector_clock import ScopedClock

B, L, H, NH, NB = 16, 256, 64, 2, 2
HS = H // NH
T = 257
ITEMNUM = 50000
NEG = -4294967295.0
EPS = 1e-8
SCALE = 1.0 / np.sqrt(HS)
NCORES = 8
BPC = B // NCORES  # batches per core
LT = L // 128      # l tiles

f32 = mybir.dt.float32
f16 = mybir.dt.float16
i16 = mybir.dt.int16
Alu = mybir.AluOpType
Act = mybir.ActivationFunctionType
AX = mybir.AxisListType


class _TC(TileContext):
    """TileContext whose tail drain splits its semaphore waits across
    multiple drain instructions (this walrus encodes one wait/inst)."""

    def _drain_and_barrier(self, tick_clock, wait_clock):
        nc = self.nc
        drain_inst = nc.sync.drain()
        wait_clock.add_sem_waits(
            drain_inst.ins, ScopedClock({None: tick_clock.global_clock})
        )
        si = drain_inst.ins.sync_info
        waits = list(si.on_wait or []) if si is not None else []
        if len(waits) > 1:
            si.on_wait = waits[:1]
            for w in waits[1:]:
                extra = nc.sync.drain()
                h = bass_rust.SemaphoreHandle(name=w.ant_name, num=w.id)
                extra.wait_op(h, w.wait_value, "sem-ge")
        nc.all_engine_barrier()
        popped = nc._tile_sem_poison_stack.pop()
        assert popped is self._sem_poison
        nc.clear_and_free_semaphores(list(self.sems.allocated().values()))
        nc.all_engine_barrier()


def _split_multi_waits(nc):
    """This walrus build encodes at most one sem wait per instruction;
    split extras onto standalone wait-only EventSemaphore instructions."""
    n = 0
    for fn in nc.m.functions:
        for bb in fn.blocks:
            insts = list(bb.instructions)
            out = []
            changed = False
            for ins in insts:
                si = ins.sync_info
                waits = list(si.on_wait) if si is not None and si.on_wait else []
                if len(waits) > 1:
                    for k, w in enumerate(waits[:-1]):
                        es = mybir.InstEventSemaphore(name=f"{ins.name}-w{k}")
                        es.engine = ins.engine
                        es.sync_info = bass_rust.SyncInfo(on_wait=[w], on_update=[])
                        out.append(es)
                        n += 1
                    si.on_wait = [waits[-1]]
                    changed = True
                out.append(ins)
            if changed:
                bb.instructions = out
    return n


def build_program():
    nc = bacc.Bacc(
        "TRN2", target_bir_lowering=False, debug=False, num_devices=NCORES
    )

    # ---------------- DRAM I/O ----------------
    d = {}
    d["seqs2"] = nc.dram_tensor("seqs2", [BPC, LT, 128, H], f32, kind="ExternalInput")
    d["sd2"] = nc.dram_tensor("sd2", [BPC, LT, 128, 1], f32, kind="ExternalInput")
    d["oht2"] = nc.dram_tensor("oht2", [BPC, LT, 128, L], f32, kind="ExternalInput")
    for nm in ("rank2", "sig2", "tgts2", "tgtl2"):
        d[nm] = nc.dram_tensor(nm, [BPC, LT, 128, L], i16, kind="ExternalInput")
    d["tgtc2"] = nc.dram_tensor("tgtc2", [BPC, LT, 128, 258], i16, kind="ExternalInput")
    d["causal2"] = nc.dram_tensor("causal2", [LT, 128, L], f32, kind="ExternalInput")
    d["ident"] = nc.dram_tensor("ident", [128, 128], f32, kind="ExternalInput")
    d["ones"] = nc.dram_tensor("ones", [1, 128], f32, kind="ExternalInput")
    d["tkT"] = nc.dram_tensor("tkT", [H, T], f32, kind="ExternalInput")
    d["tv"] = nc.dram_tensor("tv", [T, H], f32, kind="ExternalInput")
    d["apk"] = nc.dram_tensor("apk", [L, H], f32, kind="ExternalInput")
    d["apv"] = nc.dram_tensor("apv", [L, H], f32, kind="ExternalInput")
    for nm in ("qwT", "kwT", "vwT", "w1T", "w2T"):
        d[nm] = nc.dram_tensor(nm, [NB, H, H], f32, kind="ExternalInput")
    for nm in ("qb", "kb", "vb", "b1", "b2", "g1", "be1", "g2", "be2"):
        d[nm] = nc.dram_tensor(nm, [NB, H], f32, kind="ExternalInput")
    d["lg"] = nc.dram_tensor("lg", [H], f32, kind="ExternalInput")
    d["lb"] = nc.dram_tensor("lb", [H], f32, kind="ExternalInput")
    out_d = nc.dram_tensor("out2", [BPC, LT, 128, H], f32, kind="ExternalOutput")

    with _TC(nc) as tc:
        with tc.tile_pool(name="const", bufs=1) as cp, \
             tc.tile_pool(name="perb", bufs=1) as pb, \
             tc.tile_pool(name="work", bufs=2) as wk, \
             tc.tile_pool(name="hsml", bufs=2) as hp, \
             tc.tile_pool(name="psA", bufs=3, space="PSUM") as psA, \
             tc.tile_pool(name="psT", bufs=2, space="PSUM") as psT, \
             tc.tile_pool(name="psO", bufs=1, space="PSUM") as psO:

            nc.gpsimd.load_library(library_config.local_scatter)

            # ---------- constants ----------
            ident = cp.tile([128, 128], f32, tag="ident", name="ident")
            nc.sync.dma_start(out=ident[:], in_=d["ident"][:])
            ones = cp.tile([1, 128], f32, tag="ones", name="ones")
            nc.sync.dma_start(out=ones[:], in_=d["ones"][:])
            causal = cp.tile([128, LT, L], f32, tag="causal", name="causal")
            for _lt in range(LT):
                nc.sync.dma_start(out=causal[:, _lt, :], in_=d["causal2"][_lt])
            tkT = cp.tile([H, T], f32, tag="tkT", name="tkT")
            nc.sync.dma_start(out=tkT[:], in_=d["tkT"][:])

            # dV[t] = tV[t] - tV[t+1] (t<256), dV[256] = tV[256]
            tva = cp.tile([128, 2, H], f32, tag="tva", name="tva")
            tvb = cp.tile([128, 2, H], f32, tag="tvb", name="tvb")
            nc.sync.dma_start(out=tva[:], in_=d["tv"][0:256, :].rearrange("(a p) h -> p a h", p=128))
            # shifted rows 1..256
            nc.sync.dma_start(out=tvb[:, 0, :], in_=d["tv"][1:129, :])
            nc.sync.dma_start(out=tvb[:, 1, :], in_=d["tv"][129:257, :])
            dv = cp.tile([128, 2, H], f32, tag="dv", name="dv")
            nc.vector.tensor_tensor(dv[:], tva[:], tvb[:], Alu.subtract)
            # dv2 broadcast row: tV[256] -> [128, H]
            tvl = cp.tile([1, H], f32, tag="tvl", name="tvl")
            nc.sync.dma_start(out=tvl[:], in_=d["tv"][256:257, :])
            pbk = psT.tile([128, H], f32, tag="bc", name="bc")
            nc.tensor.matmul(pbk[:], ones[:], tvl[:], start=True, stop=True)
            dv2b = cp.tile([128, H], f32, tag="dv2b", name="dv2b")
            nc.scalar.copy(dv2b[:], pbk[:])

            # abs-pos tables (as K-tiles over t for the one-hot matmuls)
            apk = cp.tile([128, 2, H], f32, tag="apk", name="apk")
            nc.sync.dma_start(out=apk[:], in_=d["apk"].rearrange("(a p) h -> p a h", p=128))
            apv = cp.tile([128, 2, H], f32, tag="apv", name="apv")
            nc.sync.dma_start(out=apv[:], in_=d["apv"].rearrange("(a p) h -> p a h", p=128))

            # weights / params per block
            W = {}
            for nm in ("qwT", "kwT", "vwT", "w1T", "w2T"):
                t = cp.tile([H, NB, H], f32, tag=nm, name=nm)
                for _i in range(NB):
                    nc.sync.dma_start(out=t[:, _i, :], in_=d[nm][_i])
                W[nm] = t
            cols = {}
            for nm in ("qb", "kb"):
                t = cp.tile([H, NB], f32, tag=nm)
                nc.sync.dma_start(out=t[:], in_=d[nm].rearrange("b h -> h b"))
                cols[nm] = t
            bcast = {}
            for nm in ("vb", "b1", "b2", "g1", "be1", "g2", "be2"):
                row = cp.tile([1, NB * H], f32, tag=nm + "r")
                nc.sync.dma_start(out=row[:], in_=d[nm].rearrange("b h -> (b h)").rearrange("(o a) -> o a", o=1))
                t = cp.tile([128, NB, H], f32, tag=nm + "b")
                for i in range(NB):
                    pbc = psT.tile([128, H], f32, tag="bc", name="bc")
                    nc.tensor.matmul(pbc[:], ones[:], row[:, i * H:(i + 1) * H],
                                     start=True, stop=True)
                    nc.scalar.copy(t[:, i, :], pbc[:])
                bcast[nm] = t
            for nm in ("lg", "lb"):
                row = cp.tile([1, H], f32, tag=nm + "r")
                nc.sync.dma_start(out=row[:], in_=d[nm].rearrange("(o h) -> o h", o=1))
                pbc = psT.tile([128, H], f32, tag="bc", name="bc")
                nc.tensor.matmul(pbc[:], ones[:], row[:], start=True, stop=True)
                t = cp.tile([128, H], f32, tag=nm + "b")
                nc.scalar.copy(t[:], pbc[:])
                bcast[nm] = t

            eps_t = cp.tile([128, 1], f32, tag="eps", name="eps")
            nc.vector.memset(eps_t[:], EPS)

            def layernorm(x_tiles, g_ap, b_ap, out_tiles):
                for lt in range(LT):
                    x = x_tiles[lt]
                    s = hp.tile([128, 1], f32, tag="ln_s", name="ln_s")
                    nc.vector.tensor_reduce(s[:], x[:], AX.X, Alu.add)
                    mean = hp.tile([128, 1], f32, tag="ln_m", name="ln_m")
                    nc.vector.tensor_scalar_mul(mean[:], s[:], 1.0 / H)
                    xm = wk.tile([128, H], f32, tag="ln_xm", name="ln_xm")
                    nc.vector.tensor_scalar(xm[:], x[:], mean[:], None, Alu.subtract)
                    sq = wk.tile([128, H], f32, tag="ln_sq", name="ln_sq")
                    vs = hp.tile([128, 1], f32, tag="ln_vs", name="ln_vs")
                    nc.vector.scalar_tensor_tensor(sq[:], xm[:], 1.0, xm[:],
                                                   Alu.bypass, Alu.mult,
                                                   accum_out=vs[:])
                    sd = hp.tile([128, 1], f32, tag="ln_sd", name="ln_sd")
                    nc.scalar.activation(sd[:], vs[:], Act.Sqrt, scale=1.0 / H, bias=eps_t[:])
                    rstd = hp.tile([128, 1], f32, tag="ln_r", name="ln_r")
                    nc.vector.reciprocal(rstd[:], sd[:])
                    o = out_tiles[lt]
                    nc.vector.scalar_tensor_tensor(
                        o[:], xm[:], rstd[:], g_ap, Alu.mult, Alu.mult)
                    nc.vector.tensor_tensor(o[:], o[:], b_ap, Alu.add)

            def transpose_to(dst, src_tiles, nfree):
                """src_tiles: list of [128, nfree<=128] tiles; dst [nfree, LT*128]."""
                for lt in range(LT):
                    pt = psT.tile([128, 128], f32, tag="tp", name="tp")
                    nc.tensor.matmul(pt[:nfree, :], src_tiles[lt][:], ident[:],
                                     is_transpose=True, start=True, stop=True)
                    if lt == 0:
                        nc.scalar.copy(dst[:, lt * 128:(lt + 1) * 128], pt[:nfree, :128])
                    else:
                        nc.vector.tensor_copy(dst[:, lt * 128:(lt + 1) * 128], pt[:nfree, :128])

            # ================== per batch element ==================
            for b in range(BPC):
                X = [pb.tile([128, H], f32, tag=f"X{b}{lt}", name=f"X{b}{lt}") for lt in range(LT)]
                for lt in range(LT):
                    nc.sync.dma_start(out=X[lt][:], in_=d["seqs2"][b, lt])
                keep = []
                tlneg = []
                for lt in range(LT):
                    sd_t = hp.tile([128, 1], f32, tag="sd", name="sd")
                    nc.sync.dma_start(out=sd_t[:], in_=d["sd2"][b, lt])
                    eq = hp.tile([128, 1], f32, tag="eq", name="eq")
                    nc.vector.tensor_scalar(eq[:], sd_t[:], float(ITEMNUM - 1), None,
                                            Alu.is_equal)
                    tn = pb.tile([128, 1], f32, tag=f"tn{b}{lt}", name=f"tn{b}{lt}")
                    nc.vector.tensor_scalar_mul(tn[:], eq[:], NEG)
                    tlneg.append(tn)
                    kp = pb.tile([128, 1], f32, tag=f"kp{b}{lt}", name=f"kp{b}{lt}")
                    nc.vector.tensor_scalar(kp[:], eq[:], -1.0, 1.0, Alu.mult, Alu.add)
                    keep.append(kp)
                    nc.vector.tensor_scalar_mul(X[lt][:], X[lt][:], kp[:])

                # index tensors
                oht = pb.tile([128, LT, L], f32, tag=f"oht{b}", name=f"oht{b}")
                for _lt in range(LT):
                    nc.sync.dma_start(out=oht[:, _lt, :], in_=d["oht2"][b, _lt])
                idxt = {}
                for nm in ("rank2", "sig2", "tgts2", "tgtl2"):
                    t = pb.tile([128, LT, L], i16, tag=f"{nm}{b}", name=f"{nm}{b}")
                    for _lt in range(LT):
                        nc.sync.dma_start(out=t[:, _lt, :], in_=d[nm][b, _lt])
                    idxt[nm] = t
                tgtc = pb.tile([128, LT, 258], i16, tag=f"tgtc{b}", name=f"tgtc{b}")
                for _lt in range(LT):
                    nc.sync.dma_start(out=tgtc[:, _lt, :], in_=d["tgtc2"][b, _lt])

                # aK^T [H, L] and aV [m, H] via one-hot matmuls
                pk = psA.tile([H, L], f32, tag="akt", name="akt")
                for tt in range(2):
                    nc.tensor.matmul(pk[:], apk[:, tt, :], oht[:, tt, :],
                                     start=(tt == 0), stop=(tt == 1))
                aKT = pb.tile([H, L], f32, tag=f"aKT{b}", name=f"aKT{b}")
                nc.vector.tensor_copy(aKT[:], pk[:])
                aV = []
                for mt in range(LT):
                    pv = psT.tile([128, H], f32, tag="av", name="av")
                    for tt in range(2):
                        nc.tensor.matmul(pv[:], oht[:, tt, mt * 128:(mt + 1) * 128],
                                         apv[:, tt, :], start=(tt == 0), stop=(tt == 1))
                    av_t = pb.tile([128, H], f32, tag=f"aV{b}{mt}", name=f"aV{b}{mt}")
                    nc.vector.tensor_copy(av_t[:], pv[:])
                    aV.append(av_t)

                # ---------------- blocks ----------------
                for blk in range(NB):
                    q_in = [wk.tile([128, H], f32, tag=f"qin{lt}", name=f"qin{lt}") for lt in range(LT)]
                    layernorm(X, bcast["g1"][:, blk, :], bcast["be1"][:, blk, :], q_in)

                    qinT = wk.tile([H, L], f32, tag="qinT", name="qinT")
                    transpose_to(qinT, q_in, H)
                    XT = wk.tile([H, L], f32, tag="XT", name="XT")
                    transpose_to(XT, X, H)

                    # Q^T with bias; K'^T = K^T + kb + aK^T
                    pq = psA.tile([H, L], f32, tag="qt", name="qt")
                    nc.tensor.matmul(pq[:], W["qwT"][:, blk, :], qinT[:], start=True, stop=True)
                    QTs = wk.tile([H, L], f32, tag="QTs", name="QTs")
                    nc.scalar.activation(QTs[:], pq[:], Act.Identity,
                                         bias=cols["qb"][:, blk:blk + 1])
                    pk2 = psA.tile([H, L], f32, tag="kt", name="kt")
                    nc.tensor.matmul(pk2[:], W["kwT"][:, blk, :], XT[:], start=True, stop=True)
                    KT0 = wk.tile([H, L], f32, tag="KT0", name="KT0")
                    nc.vector.tensor_scalar(KT0[:], pk2[:],
                                            cols["kb"][:, blk:blk + 1], None,
                                            Alu.add)
                    KpT = wk.tile([H, L], f32, tag="KpT", name="KpT")
                    nc.vector.tensor_tensor(KpT[:], KT0[:], aKT[:], Alu.add)

                    # V' per m-tile
                    Vp = []
                    for mt in range(LT):
                        pv = psT.tile([128, H], f32, tag="v", name="v")
                        nc.tensor.matmul(pv[:], XT[:, mt * 128:(mt + 1) * 128],
                                         W["vwT"][:, blk, :], start=True, stop=True)
                        v_t = wk.tile([128, H], f32, tag=f"Vp{mt}", name=f"Vp{mt}")
                        nc.vector.tensor_tensor(v_t[:], pv[:], bcast["vb"][:, blk, :],
                                                Alu.add)
                        nc.vector.tensor_tensor(v_t[:], v_t[:], aV[mt][:], Alu.add)
                        Vp.append(v_t)

                    xattn = [wk.tile([128, H], f32, tag=f"xat{lt}", name=f"xat{lt}") for lt in range(LT)]
                    for h in range(NH):
                        hs = slice(h * HS, (h + 1) * HS)
                        for lt in range(LT):
                            ls = slice(lt * 128, (lt + 1) * 128)
                            # ---- time-K projection QTt [l, T] ----
                            pqt = psA.tile([128, T], f32, tag="qtt", name="qtt")
                            nc.tensor.matmul(pqt[:], QTs[hs, ls], tkT[hs, :],
                                             start=True, stop=True)
                            qttf = wk.tile([128, 258], f16, tag="qttf", name="qttf")
                            nc.vector.tensor_copy(qttf[:, 0:T], pqt[:])
                            nc.scalar.copy(qttf[:, 257:258], pqt[:, 256:257])
                            # ---- G chain ----
                            vc = wk.tile([128, L], f16, tag="vc", name="vc")
                            nc.gpsimd.local_scatter(vc[:], qttf[:], tgtc[:, lt, :],
                                                    channels=128, num_elems=L,
                                                    num_idxs=258)
                            wdf = wk.tile([128, L], f16, tag="wdf", name="wdf")
                            nc.scalar.copy(wdf[:, 0:1], vc[:, 0:1])
                            nc.vector.tensor_tensor(wdf[:, 1:L], vc[:, 1:L],
                                                    vc[:, 0:L - 1], Alu.subtract)
                            dt = wk.tile([128, L], f16, tag="dt", name="dt")
                            nc.gpsimd.local_scatter(dt[:], wdf[:], idxt["tgts2"][:, lt, :],
                                                    channels=128, num_elems=L,
                                                    num_idxs=L)
                            gs = wk.tile([128, L], f32, tag="gs", name="gs")
                            nc.vector.tensor_tensor_scan(gs[:], dt[:], dt[:], 0.0,
                                                         Alu.add, Alu.bypass)
                            gsf = wk.tile([128, L], f16, tag="gsf", name="gsf")
                            nc.scalar.copy(gsf[:], gs[:])
                            g_t = wk.tile([128, L], f16, tag="g", name="g")
                            nc.gpsimd.local_scatter(g_t[:], gsf[:], idxt["sig2"][:, lt, :],
                                                    channels=128, num_elems=L,
                                                    num_idxs=L)
                            # ---- attention weights ----
                            paw = psA.tile([128, L], f32, tag="aw", name="aw")
                            nc.tensor.matmul(paw[:], QTs[hs, ls], KpT[hs, :],
                                             start=True, stop=True)
                            aw1 = wk.tile([128, L], f32, tag="aw1", name="aw1")
                            nc.vector.scalar_tensor_tensor(aw1[:], paw[:], tlneg[lt][:],
                                                           g_t[:], Alu.add, Alu.add)
                            nc.vector.tensor_tensor(aw1[:], aw1[:], causal[:, lt, :],
                                                    Alu.add)
                            mx = hp.tile([128, 1], f32, tag="mx", name="mx")
                            nc.vector.tensor_reduce(mx[:], aw1[:], AX.X, Alu.max)
                            nb_t = hp.tile([128, 1], f32, tag="nb", name="nb")
                            nc.vector.tensor_scalar_mul(nb_t[:], mx[:], -SCALE)
                            p_t = wk.tile([128, L], f32, tag="p", name="p")
                            z_t = hp.tile([128, 1], f32, tag="z", name="z")
                            nc.scalar.activation(p_t[:], aw1[:], Act.Exp,
                                                 bias=nb_t[:], scale=SCALE,
                                                 accum_out=z_t[:])
                            r_t = hp.tile([128, 1], f32, tag="r", name="r")
                            nc.vector.reciprocal(r_t[:], z_t[:])
                            pf = wk.tile([128, L], f16, tag="pf", name="pf")
                            nc.scalar.copy(pf[:], p_t[:])
                            # ---- S chain ----
                            at = wk.tile([128, L], f16, tag="at", name="at")
                            nc.gpsimd.local_scatter(at[:], pf[:], idxt["rank2"][:, lt, :],
                                                    channels=128, num_elems=L,
                                                    num_idxs=L)
                            c2 = wk.tile([128, L], f32, tag="c2", name="c2")
                            nc.vector.tensor_tensor_scan(c2[:], at[:], at[:], 0.0,
                                                         Alu.add, Alu.bypass)
                            c2f = wk.tile([128, L], f16, tag="c2f", name="c2f")
                            nc.scalar.copy(c2f[:], c2[:])
                            cs = wk.tile([128, 258], f16, tag="cs", name="cs")
                            nc.gpsimd.local_scatter(cs[:], c2f[:], idxt["tgtl2"][:, lt, :],
                                                    channels=128, num_elems=258,
                                                    num_idxs=L)
                            rr = wk.tile([128, T], f32, tag="rr", name="rr")
                            nc.vector.tensor_tensor_scan(rr[:], cs[:, 0:T], cs[:, 0:T],
                                                         0.0, Alu.max, Alu.bypass)
                            # ---- transposes ----
                            PT = []
                            for mt in range(LT):
                                pp = psT.tile([128, 128], f32, tag="tp", name="tp")
                                nc.tensor.matmul(pp[:], p_t[:, mt * 128:(mt + 1) * 128],
                                                 ident[:], is_transpose=True,
                                                 start=True, stop=True)
                                ptsb = wk.tile([128, 128], f32, tag=f"PT{mt}", name=f"PT{mt}")
                                nc.scalar.copy(ptsb[:], pp[:])
                                PT.append(ptsb)
                            RT = []
                            for tt2 in range(2):
                                pp = psT.tile([128, 128], f32, tag="tp", name="tp")
                                nc.tensor.matmul(pp[:],
                                                 rr[:, tt2 * 128:(tt2 + 1) * 128],
                                                 ident[:], is_transpose=True,
                                                 start=True, stop=True)
                                rtsb = wk.tile([128, 128], f32, tag=f"RT{tt2}", name=f"RT{tt2}")
                                nc.scalar.copy(rtsb[:], pp[:])
                                RT.append(rtsb)
                            # ---- output accumulation ----
                            po = psO.tile([128, HS], f32, tag="o", name="o")
                            nc.tensor.matmul(po[:], PT[0][:], Vp[0][:, hs],
                                             start=True, stop=False)
                            nc.tensor.matmul(po[:], PT[1][:], Vp[1][:, hs],
                                             start=False, stop=False)
                            nc.tensor.matmul(po[:], RT[0][:], dv[:, 0, hs],
                                             start=False, stop=False)
                            nc.tensor.matmul(po[:], RT[1][:], dv[:, 1, hs],
                                             start=False, stop=True)
                            ot = wk.tile([128, HS], f32, tag="ot", name="ot")
                            nc.vector.scalar_tensor_tensor(
                                ot[:], dv2b[:, hs], rr[:, 256:257], po[:],
                                Alu.mult, Alu.add)
                            nc.vector.tensor_scalar_mul(
                                xattn[lt][:, hs], ot[:], r_t[:])

                    # residual + LN2 + FFN
                    x2 = [wk.tile([128, H], f32, tag=f"x2{lt}", name=f"x2{lt}") for lt in range(LT)]
                    for lt in range(LT):
                        nc.vector.tensor_tensor(xattn[lt][:], xattn[lt][:],
                                                q_in[lt][:], Alu.add)
                    layernorm(xattn, bcast["g2"][:, blk, :], bcast["be2"][:, blk, :], x2)
                    x2T = wk.tile([H, L], f32, tag="x2T", name="x2T")
                    transpose_to(x2T, x2, H)
                    hr = [wk.tile([128, H], f32, tag=f"hr{lt}", name=f"hr{lt}") for lt in range(LT)]
                    for lt in range(LT):
                        ph = psT.tile([128, H], f32, tag="ff", name="ff")
                        nc.tensor.matmul(ph[:], x2T[:, lt * 128:(lt + 1) * 128],
                                         W["w1T"][:, blk, :], start=True, stop=True)
                        nc.vector.tensor_tensor(hr[lt][:], ph[:],
                                                bcast["b1"][:, blk, :], Alu.add)
                        nc.vector.tensor_scalar_max(hr[lt][:], hr[lt][:], 0.0)
                    hT = wk.tile([H, L], f32, tag="hT", name="hT")
                    transpose_to(hT, hr, H)
                    for lt in range(LT):
                        ph = psT.tile([128, H], f32, tag="ff", name="ff")
                        nc.tensor.matmul(ph[:], hT[:, lt * 128:(lt + 1) * 128],
                                         W["w2T"][:, blk, :], start=True, stop=True)
                        nc.vector.tensor_tensor(X[lt][:], ph[:],
                                                bcast["b2"][:, blk, :], Alu.add)
                        nc.vector.tensor_tensor(X[lt][:], X[lt][:], x2[lt][:], Alu.add)
                        nc.vector.tensor_scalar_mul(X[lt][:], X[lt][:], keep[lt][:])

                # final layernorm + store
                fin = [wk.tile([128, H], f32, tag=f"fin{lt}", name=f"fin{lt}") for lt in range(LT)]
                layernorm(X, bcast["lg"][:], bcast["lb"][:], fin)
                for lt in range(LT):
                    nc.sync.dma_start(out=out_d[b, lt], in_=fin[lt][:])

    nc.compile()
    _split_multi_waits(nc)
    return nc


_CACHE = {}


def _host_indices_batch(tm):
    """tm [L, L] int -> (sigma, rank, tgtc, tgts, tgtl) int16 arrays."""
    sigma = np.argsort(tm, axis=1, kind="stable")
    st = np.take_along_axis(tm, sigma, axis=1)
    rank = np.empty((L, L), np.int64)
    np.put_along_axis(rank, sigma, np.arange(L)[None, :], axis=1)
    first = np.ones((L, L), bool)
    first[:, 1:] = st[:, 1:] != st[:, :-1]
    kj = np.cumsum(first, axis=1) - 1
    tgtc = np.full((L, 258), -1, np.int64)
    np.put_along_axis(tgtc[:, :T], st, kj, axis=1)
    tgts = np.full((L, L), -1, np.int64)
    rows, js = np.nonzero(first)
    tgts[rows, kj[rows, js]] = js
    last = np.ones((L, L), bool)
    last[:, :-1] = st[:, 1:] != st[:, :-1]
    tgtl = np.where(last, st, -1)
    return (sigma.astype(np.int16), rank.astype(np.int16),
            tgtc.astype(np.int16), tgts.astype(np.int16),
            tgtl.astype(np.int16))


def _tiles(a):
    """[L, X] -> [LT, 128, X]"""
    return a.reshape(LT, 128, *a.shape[1:])


def kernel(**inputs):
    inp = {k: np.asarray(v) for k, v in inputs.items()}

    if "prog" not in _CACHE:
        _CACHE["prog"] = build_program()
    nc = _CACHE["prog"]

    seqs = inp["seqs"].astype(np.float32)
    sdata = inp["seqs_data"].astype(np.int64)
    positions = inp["positions"].astype(np.int64)
    tms = inp["time_matrices"].astype(np.int64)

    causal = np.where(np.arange(L)[None, :] > np.arange(L)[:, None],
                      np.float32(NEG), np.float32(0.0))
    shared = {
        "causal2": _tiles(causal),
        "ident": np.eye(128, dtype=np.float32),
        "ones": np.ones((1, 128), np.float32),
        "tkT": np.ascontiguousarray(inp["time_K_tab"].astype(np.float32).T),
        "tv": inp["time_V_tab"].astype(np.float32),
        "apk": inp["abs_pos_K_tab"].astype(np.float32),
        "apv": inp["abs_pos_V_tab"].astype(np.float32),
        "qwT": np.ascontiguousarray(inp["Qw"].astype(np.float32).transpose(0, 2, 1)),
        "kwT": np.ascontiguousarray(inp["Kw"].astype(np.float32).transpose(0, 2, 1)),
        "vwT": np.ascontiguousarray(inp["Vw"].astype(np.float32).transpose(0, 2, 1)),
        "w1T": np.ascontiguousarray(inp["ffn_W1"].astype(np.float32).transpose(0, 2, 1)),
        "w2T": np.ascontiguousarray(inp["ffn_W2"].astype(np.float32).transpose(0, 2, 1)),
        "qb": inp["Qb"].astype(np.float32), "kb": inp["Kb"].astype(np.float32),
        "vb": inp["Vb"].astype(np.float32),
        "b1": inp["ffn_b1"].astype(np.float32), "b2": inp["ffn_b2"].astype(np.float32),
        "g1": inp["ln1_g"].astype(np.float32), "be1": inp["ln1_b"].astype(np.float32),
        "g2": inp["ln2_g"].astype(np.float32), "be2": inp["ln2_b"].astype(np.float32),
        "lg": inp["last_g"].astype(np.float32), "lb": inp["last_b"].astype(np.float32),
    }

    tidx = np.arange(L)
    in_maps = []
    for cid in range(NCORES):
        bs = [cid * BPC + i for i in range(BPC)]
        m = dict(shared)
        m["seqs2"] = np.stack([_tiles(seqs[b]) for b in bs])
        m["sd2"] = np.stack([_tiles(sdata[b].astype(np.float32)[:, None]) for b in bs])
        oht, rank2, sig2, tgts2, tgtl2, tgtc2 = [], [], [], [], [], []
        for b in bs:
            pos = positions[b]
            oh = ((pos[None, :] == tidx[:, None]) & (pos[None, :] != 0))
            oht.append(_tiles(oh.astype(np.float32)))
            sg, rk, tc, ts, tl = _host_indices_batch(tms[b])
            sig2.append(_tiles(sg)); rank2.append(_tiles(rk))
            tgtc2.append(_tiles(tc)); tgts2.append(_tiles(ts)); tgtl2.append(_tiles(tl))
        m["oht2"] = np.stack(oht)
        m["rank2"] = np.stack(rank2); m["sig2"] = np.stack(sig2)
        m["tgts2"] = np.stack(tgts2); m["tgtl2"] = np.stack(tgtl2)
        m["tgtc2"] = np.stack(tgtc2)
        in_maps.append(m)

    res = run_bass_kernel_spmd(nc, in_maps, list(range(NCORES)))
    out = np.empty((B, L, H), np.float32)
    for cid in range(NCORES):
        o = res.results[cid]["out2"]  # [BPC, LT, 128, H]
        for i in range(BPC):
            out[cid * BPC + i] = o[i].reshape(L, H)
    return out
